# revision 27
# baseline (speedup 1.0000x reference)
"""HarmonyGenerator Trainium2 kernel.

Math: the reference's 3x3 conv on [T,1,1,D] degenerates to a 3-tap conv along
the feature axis (only the kernel's middle row touches data).  Conv and the
three linear heads are both linear, so the conv folds into the head weights
(W' = 3-tap correlation of W along K) and the constant context-embedding rows
plus conv bias fold into the output bias.  The device work is one GEMM:

    out[2048, 168] = [melody | lyrics][2048, 50681] @ W'[50681, 168] + bias

Sharding: K (feature) axis split 8 ways, 6400 rows per core (zero padded).
Each core reads 1/8 of x AND 1/8 of W (~56 MB -> ~155us memory floor) and
produces a partial [168, 2048]; partials are summed on the host during the
gather/unshard step.  Matmuls run as float32r (FP22 multiply, FP32
accumulate) which streams at full PE rate for moving dims >= 256.

Device mapping per core: lhsT = W tile [128k, m<=128], rhs = xT tile
[128k, 512t] streamed, PSUM accumulates [m, 512] over 50 k-tiles for all
four 512-wide t-blocks simultaneously (8 PSUM banks).  xT is produced on the
host so every DMA is a contiguous 1 MB block.
"""

import os
import numpy as np

import concourse.bacc as bacc
import concourse.mybir as mybir
from concourse.tile import TileContext
from concourse.bass_utils import run_bass_kernel_spmd

# Problem shapes (hardcoded per contract)
T = 2048               # steps = length * 128
D_IN = 50937           # 256 ctx + 256 melody/vel + 50425 lyrics
K_GEMM = 50681         # melody(256) + lyrics(50425) features in the GEMM
N_OUT = 168            # 24 chord + 16 beat + 128 mel
N_CORES = 8
K_PER = 6400           # per-core K (8*6400 = 51200 >= 50681, zero padded)
KT = K_PER // 128      # 50 k-tiles per core
TB = 512               # t-block (max fp32 moving dim / PSUM bank)
NTB = T // TB          # 4

_NC = None
LAST_RESULT = None     # BassKernelResults of the most recent run (for test.py)

# Matmul input dtype: fp16 (half the x DMA traffic, ~4e-4 rel err) or
# f32r (fp32 bytes, FP22 multiply, ~2e-4 rel err).
DTYPE = os.environ.get("HARMONY_DTYPE", "fp16")


def _in_dt():
    return mybir.dt.float16 if DTYPE == "fp16" else mybir.dt.float32r


def _np_in_dt():
    return np.float16 if DTYPE == "fp16" else np.float32


def _build_nc():
    f32 = mybir.dt.float32
    fin = _in_dt()
    nc = bacc.Bacc()
    xt = nc.dram_tensor("xt", [K_PER, T], fin, kind="ExternalInput")
    w = nc.dram_tensor("w", [128, KT * N_OUT], fin, kind="ExternalInput")
    out = nc.dram_tensor("out", [N_OUT, T], f32, kind="ExternalOutput")

    # k-tiles per DMA chunk: small head chunks so the first matmul fires
    # early, large tail chunks for DMA efficiency (2-2.5 MB fp16)
    X_SCHED = [1, 1] + [2] * 24
    W_SCHED = [2, 4, 8, 12, 12, 12]
    assert sum(X_SCHED) == KT and sum(W_SCHED) == KT
    with TileContext(nc) as tc:
        with (
            tc.tile_pool(name="wp", bufs=1) as wp,
            tc.tile_pool(name="xp", bufs=10) as xp,
            tc.tile_pool(name="op", bufs=4) as op,
            tc.tile_pool(name="ps", bufs=1, space="PSUM") as ps,
        ):
            # W preloaded in independent chunks so the first matmuls don't
            # wait on the whole 2-4 MB weight transfer.
            # HAM warm-up: the PE clock-gate holds matmuls at 1.2 GHz until
            # ~3.4us of sustained activity.  Burn the DMA-fill window (no real
            # operands on chip yet) on dummy matmuls so real MMs start at
            # 2.4 GHz.  Scratch PSUM bank; results never read.
            dm = wp.tile([128, TB], fin, tag="warm", name="warmup")
            nc.gpsimd.memset(dm[:], 0.0)
            ps_warm = ps.tile([128, TB], f32, tag="warm_ps", name="ps_warm")
            for _ in range(10):
                nc.tensor.matmul(ps_warm[:], dm[:, 0:128], dm[:], start=True, stop=True)

            # W loads on the gpsimd SWDGE ring, leaving both HWDGE rings
            # (sync + scalar) free to alternate x chunks.
            # w_of[kt] -> (tile, col offset of that k-tile's weights)
            w_of = {}
            kt0 = 0
            for wc, n in enumerate(W_SCHED):
                wt = wp.tile([128, n * N_OUT], fin, tag=f"w{wc}", name=f"w{wc}")
                nc.gpsimd.dma_start(wt[:], w[:, kt0 * N_OUT:(kt0 + n) * N_OUT])
                for j in range(n):
                    w_of[kt0 + j] = (wt, j * N_OUT)
                kt0 += n

            # Persistent accumulators: 4 mel banks + 2 shared cb banks.  Each
            # cb bank holds two t-blocks' [40, TB] outputs col-tiled into
            # partitions 0:40 and 64:104 (concurrent matmuls via tile_position).
            psm = [ps.tile([128, TB], f32, tag=f"m{t}", name=f"psm{t}") for t in range(NTB)]
            psc = [ps.tile([128, TB], f32, tag=f"c{p}", name=f"psc{p}") for p in range(NTB // 2)]

            xc0 = 0
            for xc, xn in enumerate(X_SCHED):
                x_tile = xp.tile([128, xn * T], fin, tag="x", name="x_tile")
                ring = nc.sync if xc % 2 == 0 else nc.scalar
                if xn == 1:
                    ring.dma_start(x_tile[:], xt[xc0 * 128:(xc0 + 1) * 128, :])
                else:
                    ring.dma_start(
                        x_tile[:].rearrange("p (a t) -> p a t", a=xn),
                        xt[xc0 * 128:(xc0 + xn) * 128, :].rearrange(
                            "(a p) t -> p a t", p=128
                        ),
                    )
                for a in range(xn):
                    kt = xc0 + a
                    wt, j = w_of[kt]
                    lhs_m = wt[:, j: j + 128]
                    lhs_c = wt[:, j + 128: j + N_OUT]
                    first, last = kt == 0, kt == KT - 1

                    def rhs_of(t):
                        return x_tile[:, a * T + t * TB: a * T + (t + 1) * TB]

                    def cb_pair(p):
                        # two concurrent 40-col matmuls in distinct col groups
                        nc.tensor.matmul(psc[p][0:40, :], lhs_c, rhs_of(2 * p),
                                         start=first, stop=last, tile_position=(0, 0))
                        nc.tensor.matmul(psc[p][64:104, :], lhs_c, rhs_of(2 * p + 1),
                                         start=first, stop=last, tile_position=(0, 64))

                    if not last:
                        # group by stationary operand: 4 mel MMs, then cb pairs
                        for t in range(NTB):
                            nc.tensor.matmul(psm[t][:], lhs_m, rhs_of(t), start=first, stop=last)
                        cb_pair(0)
                        cb_pair(1)
                    else:
                        # final k-tile: finish banks in eviction order so PSUM
                        # evictions start while remaining MMs run
                        nc.tensor.matmul(psm[0][:], lhs_m, rhs_of(0), start=first, stop=last)
                        nc.tensor.matmul(psm[1][:], lhs_m, rhs_of(1), start=first, stop=last)
                        cb_pair(0)
                        nc.tensor.matmul(psm[2][:], lhs_m, rhs_of(2), start=first, stop=last)
                        nc.tensor.matmul(psm[3][:], lhs_m, rhs_of(3), start=first, stop=last)
                        cb_pair(1)
                xc0 += xn

            for t in range(NTB):
                o1 = op.tile([128, TB], f32, tag="o1", name="o1")
                nc.vector.tensor_copy(o1[:], psm[t][:])
                nc.sync.dma_start(out[0:128, t * TB:(t + 1) * TB], o1[:])
            for p in range(NTB // 2):
                o2 = op.tile([104, TB], f32, tag="o2", name="o2")
                nc.vector.tensor_copy(o2[:], psc[p][0:104, :])
                nc.sync.dma_start(out[128:N_OUT, 2 * p * TB:(2 * p + 1) * TB], o2[0:40, :])
                nc.sync.dma_start(out[128:N_OUT, (2 * p + 1) * TB:(2 * p + 2) * TB], o2[64:104, :])
    return nc


def _get_nc():
    global _NC
    if _NC is None:
        _NC = _build_nc()
        if not _NC.is_finalized():
            _NC.finalize()
    return _NC


def kernel(**inputs):
    global LAST_RESULT
    melody = np.ascontiguousarray(np.asarray(inputs["melody_tensor"], dtype=np.float32))
    lyrics = np.ascontiguousarray(np.asarray(inputs["lyrics_tensor"], dtype=np.float32))
    emb = np.asarray(inputs["emb"], dtype=np.float32)
    conv_w = np.asarray(inputs["conv_w"], dtype=np.float32)
    conv_b = np.asarray(inputs["conv_b"], dtype=np.float32)
    w_chord = np.asarray(inputs["w_chord"], dtype=np.float32)
    w_beat = np.asarray(inputs["w_beat"], dtype=np.float32)
    w_mel = np.asarray(inputs["w_mel"], dtype=np.float32)
    b_heads = np.concatenate([
        np.asarray(inputs["b_chord"], dtype=np.float32),
        np.asarray(inputs["b_beat"], dtype=np.float32),
        np.asarray(inputs["b_mel"], dtype=np.float32),
    ])
    genre = int(np.asarray(inputs["genre"]).reshape(-1)[0])
    tempo = int(np.asarray(inputs["tempo"]).reshape(-1)[0])
    key_sig = int(np.asarray(inputs["key_sig"]).reshape(-1)[0])

    # Fold conv into head weights: W'[e] = k0*W[e+1] + k1*W[e] + k2*W[e-1]
    W = np.concatenate([w_chord, w_beat, w_mel], axis=1)  # [50937, 168]
    k0, k1, k2 = (float(v) for v in conv_w[0, 0, 1, :])
    Wp = k1 * W
    Wp[:-1] += k0 * W[1:]
    Wp[1:] += k2 * W[:-1]

    # Bias: head biases + conv bias * colsum(W) + context-embedding term
    ids = [genre, 10 + tempo, 20 + key_sig, 34]
    ctx = emb[ids].sum(axis=0).astype(np.float64)  # [256]
    bias = (
        b_heads.astype(np.float64)
        + float(conv_b[0]) * W.sum(axis=0, dtype=np.float64)
        + ctx @ Wp[0:256].astype(np.float64)
    )  # [168]

    # Device operands: xT [51200, 2048] (zero padded), W' rows 256.. packed
    np_dt = _np_in_dt()
    K_PAD = N_CORES * K_PER
    XT = np.zeros((K_PAD, T), np_dt)
    XT[0:256] = melody.T
    XT[256:K_GEMM] = lyrics.T
    Wg = np.zeros((K_PAD, N_OUT), np_dt)
    Wg[0:K_GEMM] = Wp[256:]

    in_maps = []
    for c in range(N_CORES):
        wc = (
            Wg[c * K_PER:(c + 1) * K_PER]
            .reshape(KT, 128, N_OUT)
            .transpose(1, 0, 2)
            .reshape(128, KT * N_OUT)
        )
        in_maps.append({
            "xt": XT[c * K_PER:(c + 1) * K_PER],
            "w": np.ascontiguousarray(wc),
        })

    trace = bool(os.environ.get("HARMONY_TRACE"))
    res = run_bass_kernel_spmd(_get_nc(), in_maps, core_ids=list(range(N_CORES)), trace=trace)
    LAST_RESULT = res

    acc = np.zeros((N_OUT, T), np.float64)
    for r in res.results:
        acc += r["out"]
    out = (acc + bias[:, None]).T
    return np.ascontiguousarray(out.astype(np.float32))
